# revision 27
# baseline (speedup 1.0000x reference)
"""Trainium2 Bass kernel for the ACF (Attentive Collaborative Filtering) model.

Strategy (8 NeuronCores, positive-item axis P=512 sharded 64 items/core):
  - The only heavy compute is f_u_i_pos @ [Wi0_ix | Wc0_i]  ([3136,2048]@[2048,128]
    per core).  f is shipped per-core as fp8-e4m3, pre-transposed and packed
    row-tile-major on the host so every DMA is contiguous and the contraction
    dim (features) lands on SBUF partitions.  Wcat is pre-scaled by 32 to dodge
    fp8 subnormals; the inverse scale is folded into the relu activation scale
    and the ginv broadcast constant (zero extra instructions).
  - One fused PE pass produces X^T = (f@Wi0_ix)^T on PSUM partitions 0-63 and
    H^T = (f@Wc0_i)^T on partitions 64-127.  Region softmax (over R=49) and the
    b-weighted sum over regions collapse to tiny per-core ops because
    all_x @ Wi0_ix == sum_r b*(f@Wi0_ix) -- all_x itself is never materialized.
  - Row tiles and epilogue tiles coincide (9 items per tile, plus a 1-item
    final tile so the unavoidable serial dependency chain at the end operates
    on minimum-size tensors).  The item-level attention (relu, Wi1 logit, exp,
    exp-weighted p_pos accumulation into a single PSUM bank) is also done
    per-tile, so it pipelines behind later tiles' matmuls.
  - Item-level attention partials (sum of exp, exp-weighted p_pos sum) are
    DMA'd out per core; the cross-shard softmax combine (exact: no max shift is
    used anywhere, and softmax is shift-invariant so bc1/bi1 cancel) and the
    final tiny dot products happen on the host during unsharding.
"""

import numpy as np

import concourse.bass as bass
import concourse.bacc as bacc
import concourse.tile as tile
from concourse import mybir
from concourse.bass_utils import run_bass_kernel_spmd

NCORES = 8
P_FULL = 512
R = 49
FEAT = 2048
D = 64
F = 200
P_LOC = P_FULL // NCORES      # 64 items per core
ROWS = P_LOC * R              # 3136 (item, region) rows per core
KC = FEAT // 128              # 16 contraction chunks of 128
RTS = [9 * R] * 7 + [R]       # row tiles (items*49): 7x441 + 1x49
NT = len(RTS)
OFFS = [sum(RTS[:i]) for i in range(len(RTS))]
DMA_RTS = [441, 882, 882, 882, 49]   # host packing granularity = DMA groups
DMA_OFFS = [sum(DMA_RTS[:i]) for i in range(len(DMA_RTS))]
WSCALE = 32.0                 # fp8 pre-scale on Wcat (folded back out downstream)

_CACHE: dict = {}


def _build():
    dt = mybir.dt
    AF = mybir.ActivationFunctionType
    nc = bacc.Bacc()

    ft = nc.declare_dram_parameter("ft", [128, KC * ROWS], dt.float8e4, isOutput=False)
    wcat = nc.declare_dram_parameter("wcat", [128, KC * 128], dt.float8e4, isOutput=False)
    uc = nc.declare_dram_parameter("uc", [D, 1], dt.float32, isOutput=False)
    wc1t = nc.declare_dram_parameter("wc1t", [D, D], dt.float16, isOutput=False)
    et = nc.declare_dram_parameter("et", [D, P_LOC], dt.float32, isOutput=False)
    wi1 = nc.declare_dram_parameter("wi1", [D, 1], dt.float32, isOutput=False)
    ppos = nc.declare_dram_parameter("ppos", [P_LOC, F + 1], dt.float32, isOutput=False)
    out = nc.declare_dram_parameter("out", [1, F + 1], dt.float32, isOutput=True)

    with tile.TileContext(nc) as tc:
        with (
            tc.tile_pool(name="singles", bufs=1) as singles,
            tc.tile_pool(name="ftp", bufs=3) as ftp,
            tc.tile_pool(name="eep", bufs=3) as eep,
            tc.tile_pool(name="xap", bufs=3) as xap,
            tc.tile_pool(name="ebp", bufs=4) as ebp,
            tc.tile_pool(name="hrp", bufs=3) as hrp,
            tc.tile_pool(name="tmpp", bufs=3) as tmpp,
            tc.tile_pool(name="finp", bufs=2) as finp,
            tc.tile_pool(name="psbig", bufs=2, space="PSUM") as psbig,
            tc.tile_pool(name="pssmall", bufs=3, space="PSUM") as pssmall,
            tc.tile_pool(name="psvp", bufs=1, space="PSUM") as psvp,
        ):
            # matmul #1 needs ft tile-0 quarter 0 and wcat chunks 0-3; order the
            # sync-queue DMAs so those land first.
            ft0 = ftp.tile([128, KC, RTS[0]], dt.float8e4, tag="ft0")
            q = KC // 4
            nc.sync.dma_start(out=ft0[:, 0:q], in_=ft[:, 0 : q * RTS[0]])
            wcat_sb = singles.tile([128, KC, 128], dt.float8e4)
            wcv = wcat.rearrange("p (k m) -> p k m", k=KC)
            nc.sync.dma_start(out=wcat_sb[:, 0:q], in_=wcv[:, 0:q])
            nc.sync.dma_start(out=wcat_sb[:, q:], in_=wcv[:, q:])
            for qi in range(1, 4):
                nc.sync.dma_start(
                    out=ft0[:, qi * q : (qi + 1) * q],
                    in_=ft[:, qi * q * RTS[0] : (qi + 1) * q * RTS[0]],
                )

            # small constants on the gpsimd (SWDGE) queue so they don't delay ft
            ucw_sb = singles.tile([128, 1], dt.float32)
            nc.gpsimd.dma_start(out=ucw_sb[D : 2 * D, :], in_=uc[:])
            wc1t_sb = singles.tile([128, D], dt.float16)
            nc.gpsimd.dma_start(out=wc1t_sb[D : 2 * D, :], in_=wc1t[:])
            et_sb = singles.tile([D, P_LOC], dt.float32)
            nc.gpsimd.dma_start(out=et_sb, in_=et[:])
            wi1_sb = singles.tile([D, 1], dt.float32)
            nc.gpsimd.dma_start(out=wi1_sb, in_=wi1[:])
            ppos_sb = singles.tile([P_LOC, F + 1], dt.float32)
            nc.gpsimd.dma_start(out=ppos_sb, in_=ppos[:])
            # item 63 (the 1-item last tile) needs its own base-0 copy
            ppos63_sb = singles.tile([1, F + 1], dt.float32)
            nc.gpsimd.dma_start(out=ppos63_sb, in_=ppos[P_LOC - 1 : P_LOC])

            ones16_sb = singles.tile([1, P_LOC], dt.float16)
            nc.vector.memset(ones16_sb, 1.0)
            ones32_sb = singles.tile([1, P_LOC], dt.float32)
            nc.vector.memset(ones32_sb, 1.0 / WSCALE)

            atA_sb = singles.tile([P_LOC, P_LOC - 1], dt.float32)
            gA_sb = singles.tile([1, P_LOC - 1], dt.float32)
            psv = psvp.tile([1, F + 1], dt.float32)

            # ft DMAs are merged in pairs of row tiles (bigger transfers keep
            # the 16 SDMA engines at line rate); processing stays per-tile.
            DMA_GROUPS = [[0], [1, 2], [3, 4], [5, 6], [7]]
            for grp in DMA_GROUPS:
                if grp == [0]:
                    ftg = ft0
                else:
                    w = sum(RTS[t] for t in grp)
                    ftg = ftp.tile([128, KC, w], dt.float8e4, tag="ftg")
                    nc.sync.dma_start(
                        out=ftg,
                        in_=ft[:, KC * OFFS[grp[0]] : KC * (OFFS[grp[0]] + w)],
                    )
                for t in grp:
                    rt = RTS[t]
                    lo = OFFS[t] - OFFS[grp[0]]
                    ipt = rt // R
                    ps = psbig.tile([128, rt], dt.float32, tag="ps")
                    for k in range(KC):
                        nc.tensor.matmul(
                            ps,
                            wcat_sb[:, k],
                            ftg[:, k, lo : lo + rt],
                            start=(k == 0),
                            stop=(k == KC - 1),
                        )
                    # H^T slice (partitions 64-127): relu(H/WSCALE + uc), f16
                    hr = hrp.tile([128, rt], dt.float16, tag="hr")
                    nc.scalar.activation(
                        hr[D : 2 * D], ps[D : 2 * D], AF.Relu,
                        bias=ucw_sb[D : 2 * D, 0:1], scale=1.0 / WSCALE,
                    )
                    # region logits broadcast to 64 partitions in one matmul:
                    # wc1t is Wc1 replicated across 64 output columns, so every
                    # output partition carries the same logit row.  exp then
                    # lands already-broadcast (and runs on 64 ACT lanes).
                    psl = pssmall.tile([P_LOC, rt], dt.float32, tag="small")
                    nc.tensor.matmul(psl, wc1t_sb[D : 2 * D, :], hr[D : 2 * D])
                    eb_t = ebp.tile([P_LOC, rt], dt.float16, tag="ebt")
                    nc.scalar.activation(eb_t, psl, AF.Exp)

                    # X^T slice (partitions 0-63) -> SBUF (exact f32 copy;
                    # the scalar engine's Copy path is low-precision)
                    xa_t = xap.tile([P_LOC, rt], dt.float32, tag="xat")
                    nc.vector.tensor_copy(xa_t, ps[0:D])

                    #   at[:, p] = sum_r e[p,r] * X^T[:, (p,r)], g[p] = sum_r e
                    tmp = tmpp.tile([P_LOC, rt], dt.float32, tag="tmp")
                    nc.vector.tensor_mul(tmp, xa_t, eb_t)
                    if t < NT - 1:
                        at_out = atA_sb[:, t * 9 : t * 9 + ipt]
                        g_out = gA_sb[0:1, t * 9 : t * 9 + ipt]
                    else:
                        at7 = finp.tile([P_LOC, 1], dt.float32, tag="at7")
                        g7 = finp.tile([1, 1], dt.float32, tag="g7")
                        at_out, g_out = at7, g7
                    nc.vector.tensor_reduce(
                        out=at_out,
                        in_=tmp.rearrange("p (i r) -> p i r", r=R),
                        axis=mybir.AxisListType.X,
                        op=mybir.AluOpType.add,
                    )
                    nc.vector.tensor_reduce(
                        out=g_out,
                        in_=eb_t[0:1, :].rearrange("a (i r) -> a i r", r=R),
                        axis=mybir.AxisListType.X,
                        op=mybir.AluOpType.add,
                    )

                    # item-level attention, batched: items 0-62 fire right
                    # after tile 6 (overlapping tile 7's matmuls); item 63's
                    # minimum-size chain is the only serial tail.
                    #   s = relu(E^T + A^T/(G*WSCALE)); l = s.Wi1
                    #   psv += exp(l) @ [p_pos | 1]
                    if t >= NT - 2:
                        first = t == NT - 2
                        nit = P_LOC - 1 if first else 1
                        at_in = atA_sb if first else at7
                        g_in = gA_sb if first else g7
                        rhs_pp = ppos_sb[0 : P_LOC - 1] if first else ppos63_sb
                        e_in = et_sb[:, 0 : P_LOC - 1] if first else et_sb[:, P_LOC - 1 : P_LOC]
                        ginv_t = finp.tile([1, nit], dt.float32, tag="ginv")
                        nc.vector.reciprocal(ginv_t, g_in)
                        psg = pssmall.tile([D, nit], dt.float32, tag="small")
                        nc.tensor.matmul(psg, ones32_sb[0:1, 0:D], ginv_t)
                        st_t = finp.tile([D, nit], dt.float32, tag="stt")
                        nc.vector.tensor_mul(st_t, at_in, psg)
                        nc.vector.tensor_add(st_t, st_t, e_in)
                        nc.vector.tensor_relu(st_t, st_t)
                        psa = pssmall.tile([nit, 1], dt.float32, tag="small")
                        nc.tensor.matmul(psa, st_t, wi1_sb)
                        ecol_t = finp.tile([nit, 1], dt.float32, tag="ecol")
                        nc.scalar.activation(ecol_t, psa, AF.Exp)
                        nc.tensor.matmul(
                            psv, ecol_t, rhs_pp, start=first, stop=not first
                        )

            out_sb = singles.tile([1, F + 1], dt.float32)
            nc.vector.tensor_copy(out_sb, psv)
            nc.sync.dma_start(out=out[:], in_=out_sb)

    nc.finalize()
    return nc


def _get_nc():
    if "nc" not in _CACHE:
        _CACHE["nc"] = _build()
    return _CACHE["nc"]


def kernel(**inputs) -> np.ndarray:
    f = np.asarray(inputs["f_u_i_pos"], dtype=np.float32)
    Gu = np.asarray(inputs["Gu"], np.float32)
    Gi = np.asarray(inputs["Gi"], np.float32)
    Pi = np.asarray(inputs["Pi"], np.float32)
    user = int(np.asarray(inputs["user"]))
    item = int(np.asarray(inputs["item"]))
    pos = np.asarray(inputs["user_pos"]).reshape(-1).astype(np.int64)

    g_u = Gu[user]
    gamma_i = Gi[item]
    p_i_item = Pi[item]
    g_pos = Gi[pos]
    p_pos = Pi[pos]

    uc = g_u @ np.asarray(inputs["Wc0_u"], np.float32) + np.asarray(inputs["bc0"], np.float32)
    ui = g_u @ np.asarray(inputs["Wi0_u"], np.float32) + np.asarray(inputs["bi0"], np.float32)
    E = (
        ui[None]
        + g_pos @ np.asarray(inputs["Wi0_iv"], np.float32)
        + p_pos @ np.asarray(inputs["Wi0_ip"], np.float32)
    )  # [512, 64]
    wc1 = np.asarray(inputs["Wc1"], np.float32).reshape(D, 1)
    wi1 = np.asarray(inputs["Wi1"], np.float32).reshape(D, 1)
    Wcat = np.concatenate(
        [np.asarray(inputs["Wi0_ix"], np.float32), np.asarray(inputs["Wc0_i"], np.float32)],
        axis=1,
    )  # [2048, 128]

    # Host packing: SBUF-layout-exact, so every device DMA is contiguous.
    import ml_dtypes

    f8 = ml_dtypes.float8_e4m3
    wcat_host = np.ascontiguousarray(
        (Wcat * WSCALE).reshape(KC, 128, 128).transpose(1, 0, 2).reshape(128, KC * 128)
    ).astype(f8)

    f8_rows = f[0].reshape(P_FULL * R, FEAT).astype(f8)  # [25088, 2048]

    in_maps = []
    for c in range(NCORES):
        fc_t = f8_rows[c * ROWS : (c + 1) * ROWS].T  # [2048, 3136] (view)
        a = fc_t.reshape(KC, 128, ROWS)
        ft_host = np.concatenate(
            [
                a[:, :, o : o + rt].transpose(1, 0, 2).reshape(128, KC * rt)
                for o, rt in zip(DMA_OFFS, DMA_RTS)
            ],
            axis=1,
        )  # [128, KC*ROWS], DMA-group-major, contiguous per partition slice
        ppos_ext = np.concatenate(
            [p_pos[c * P_LOC : (c + 1) * P_LOC], np.ones((P_LOC, 1), np.float32)], axis=1
        ).astype(np.float32)  # [64, 201]
        in_maps.append(
            {
                "ft": np.ascontiguousarray(ft_host),
                "wcat": wcat_host,
                "uc": uc.reshape(D, 1).astype(np.float32),
                "wc1t": np.ascontiguousarray(np.repeat(wc1, D, axis=1)).astype(np.float16),
                "et": np.ascontiguousarray(E[c * P_LOC : (c + 1) * P_LOC].T).astype(
                    np.float32
                ),
                "wi1": wi1,
                "ppos": ppos_ext,
            }
        )

    nc = _get_nc()
    _CACHE["in_maps"] = in_maps
    res = run_bass_kernel_spmd(nc, in_maps, core_ids=list(range(NCORES)))
    outs = [np.asarray(res.results[c]["out"][0], np.float64) for c in range(NCORES)]
    _CACHE["last_outs"] = outs

    V = sum(o[:F] for o in outs)
    S = sum(float(o[F]) for o in outs)
    all_a = V / S
    xui = np.float32(np.dot(g_u.astype(np.float64) + all_a, gamma_i.astype(np.float64)))
    return (np.array(xui, np.float32), g_u, gamma_i, p_i_item)


# revision 28
# speedup vs baseline: 1.1078x; 1.1078x over previous
"""Trainium2 Bass kernel for the ACF (Attentive Collaborative Filtering) model.

Strategy (8 NeuronCores, positive-item axis P=512 sharded 64 items/core):
  - The only heavy compute is f_u_i_pos @ [Wi0_ix | Wc0_i]  ([3136,2048]@[2048,128]
    per core).  f is shipped per-core as fp8-e4m3, pre-transposed and packed
    row-tile-major on the host so every DMA is contiguous and the contraction
    dim (features) lands on SBUF partitions.  Wcat is pre-scaled by 32 to dodge
    fp8 subnormals; the inverse scale is folded into the relu activation scale
    and the ginv broadcast constant (zero extra instructions).
  - One fused PE pass produces X^T = (f@Wi0_ix)^T on PSUM partitions 0-63 and
    H^T = (f@Wc0_i)^T on partitions 64-127.  Region softmax (over R=49) and the
    b-weighted sum over regions collapse to tiny per-core ops because
    all_x @ Wi0_ix == sum_r b*(f@Wi0_ix) -- all_x itself is never materialized.
  - Row tiles and epilogue tiles coincide (9 items per tile, plus a 1-item
    final tile so the unavoidable serial dependency chain at the end operates
    on minimum-size tensors).  The item-level attention (relu, Wi1 logit, exp,
    exp-weighted p_pos accumulation into a single PSUM bank) is also done
    per-tile, so it pipelines behind later tiles' matmuls.
  - Item-level attention partials (sum of exp, exp-weighted p_pos sum) are
    DMA'd out per core; the cross-shard softmax combine (exact: no max shift is
    used anywhere, and softmax is shift-invariant so bc1/bi1 cancel) and the
    final tiny dot products happen on the host during unsharding.
"""

import numpy as np

import concourse.bass as bass
import concourse.bacc as bacc
import concourse.tile as tile
from concourse import mybir
from concourse.bass_utils import run_bass_kernel_spmd

NCORES = 8
P_FULL = 512
R = 49
FEAT = 2048
D = 64
F = 200
P_LOC = P_FULL // NCORES      # 64 items per core
ROWS = P_LOC * R              # 3136 (item, region) rows per core
KC = FEAT // 128              # 16 contraction chunks of 128
RTS = [9 * R] * 7 + [R]       # row tiles (items*49): 7x441 + 1x49
NT = len(RTS)
OFFS = [sum(RTS[:i]) for i in range(len(RTS))]
DMA_RTS = [441, 882, 882, 882, 49]   # host packing granularity = DMA groups
DMA_OFFS = [sum(DMA_RTS[:i]) for i in range(len(DMA_RTS))]
WSCALE = 32.0                 # fp8 pre-scale on Wcat (folded back out downstream)

_CACHE: dict = {}


def _build():
    dt = mybir.dt
    AF = mybir.ActivationFunctionType
    nc = bacc.Bacc()

    ft = nc.declare_dram_parameter("ft", [128, KC * ROWS], dt.float8e4, isOutput=False)
    wcat = nc.declare_dram_parameter("wcat", [128, KC * 128], dt.float8e4, isOutput=False)
    uc = nc.declare_dram_parameter("uc", [D, 1], dt.float32, isOutput=False)
    wc1t = nc.declare_dram_parameter("wc1t", [D, D], dt.float16, isOutput=False)
    et = nc.declare_dram_parameter("et", [D, P_LOC], dt.float32, isOutput=False)
    wi1 = nc.declare_dram_parameter("wi1", [D, 1], dt.float32, isOutput=False)
    ppos = nc.declare_dram_parameter("ppos", [P_LOC, F + 1], dt.float32, isOutput=False)
    out = nc.declare_dram_parameter("out", [1, F + 1], dt.float32, isOutput=True)

    with tile.TileContext(nc) as tc:
        with (
            tc.tile_pool(name="singles", bufs=1) as singles,
            tc.tile_pool(name="ftp", bufs=3) as ftp,
            tc.tile_pool(name="eep", bufs=3) as eep,
            tc.tile_pool(name="xap", bufs=3) as xap,
            tc.tile_pool(name="ebp", bufs=3) as ebp,
            tc.tile_pool(name="hrp", bufs=2) as hrp,
            tc.tile_pool(name="tmpp", bufs=2) as tmpp,
            tc.tile_pool(name="finp", bufs=2) as finp,
            tc.tile_pool(name="psbig", bufs=2, space="PSUM") as psbig,
            tc.tile_pool(name="pssmall", bufs=2, space="PSUM") as pssmall,
            tc.tile_pool(name="psvp", bufs=1, space="PSUM") as psvp,
        ):
            # matmul #1 needs ft tile-0 quarter 0 and wcat chunks 0-3; order the
            # sync-queue DMAs so those land first.
            ft0 = ftp.tile([128, KC, RTS[0]], dt.float8e4, tag="ft0")
            q = KC // 4
            nc.sync.dma_start(out=ft0[:, 0:q], in_=ft[:, 0 : q * RTS[0]])
            wcat_sb = singles.tile([128, KC, 128], dt.float8e4)
            wcv = wcat.rearrange("p (k m) -> p k m", k=KC)
            nc.sync.dma_start(out=wcat_sb[:, 0:q], in_=wcv[:, 0:q])
            nc.sync.dma_start(out=wcat_sb[:, q:], in_=wcv[:, q:])
            for qi in range(1, 4):
                nc.sync.dma_start(
                    out=ft0[:, qi * q : (qi + 1) * q],
                    in_=ft[:, qi * q * RTS[0] : (qi + 1) * q * RTS[0]],
                )

            # small constants on the gpsimd (SWDGE) queue so they don't delay ft
            ucw_sb = singles.tile([128, 1], dt.float32)
            nc.gpsimd.dma_start(out=ucw_sb[D : 2 * D, :], in_=uc[:])
            wc1t_sb = singles.tile([128, D], dt.float16)
            nc.gpsimd.dma_start(out=wc1t_sb[D : 2 * D, :], in_=wc1t[:])
            et_sb = singles.tile([D, P_LOC], dt.float32)
            nc.gpsimd.dma_start(out=et_sb, in_=et[:])
            wi1_sb = singles.tile([D, 1], dt.float32)
            nc.gpsimd.dma_start(out=wi1_sb, in_=wi1[:])
            ppos_sb = singles.tile([P_LOC, F + 1], dt.float32)
            nc.gpsimd.dma_start(out=ppos_sb, in_=ppos[:])
            # item 63 (the 1-item last tile) needs its own base-0 copy
            ppos63_sb = singles.tile([1, F + 1], dt.float32)
            nc.gpsimd.dma_start(out=ppos63_sb, in_=ppos[P_LOC - 1 : P_LOC])

            ones16_sb = singles.tile([1, P_LOC], dt.float16)
            nc.vector.memset(ones16_sb, 1.0)
            ones32_sb = singles.tile([1, P_LOC], dt.float32)
            nc.vector.memset(ones32_sb, 1.0 / WSCALE)

            atA_sb = singles.tile([P_LOC, P_LOC - 1], dt.float32)
            gA_sb = singles.tile([1, P_LOC - 1], dt.float32)
            psv = psvp.tile([1, F + 1], dt.float32)

            # ft DMAs are merged in pairs of row tiles (bigger transfers keep
            # the 16 SDMA engines at line rate); processing stays per-tile.
            DMA_GROUPS = [[0], [1, 2], [3, 4], [5, 6], [7]]
            for grp in DMA_GROUPS:
                if grp == [0]:
                    ftg = ft0
                else:
                    w = sum(RTS[t] for t in grp)
                    ftg = ftp.tile([128, KC, w], dt.float8e4, tag="ftg")
                    nc.sync.dma_start(
                        out=ftg,
                        in_=ft[:, KC * OFFS[grp[0]] : KC * (OFFS[grp[0]] + w)],
                    )
                for t in grp:
                    rt = RTS[t]
                    lo = OFFS[t] - OFFS[grp[0]]
                    ipt = rt // R
                    ps = psbig.tile([128, rt], dt.float32, tag="ps")
                    for k in range(KC):
                        nc.tensor.matmul(
                            ps,
                            wcat_sb[:, k],
                            ftg[:, k, lo : lo + rt],
                            start=(k == 0),
                            stop=(k == KC - 1),
                        )
                    # H^T slice (partitions 64-127): relu(H/WSCALE + uc), f16
                    hr = hrp.tile([128, rt], dt.float16, tag="hr")
                    nc.scalar.activation(
                        hr[D : 2 * D], ps[D : 2 * D], AF.Relu,
                        bias=ucw_sb[D : 2 * D, 0:1], scale=1.0 / WSCALE,
                    )
                    # region logits broadcast to 64 partitions in one matmul:
                    # wc1t is Wc1 replicated across 64 output columns, so every
                    # output partition carries the same logit row.  exp then
                    # lands already-broadcast (and runs on 64 ACT lanes).
                    psl = pssmall.tile([P_LOC, rt], dt.float32, tag="small")
                    nc.tensor.matmul(psl, wc1t_sb[D : 2 * D, :], hr[D : 2 * D])
                    eb_t = ebp.tile([P_LOC, rt], dt.float16, tag="ebt")
                    nc.scalar.activation(eb_t, psl, AF.Exp)

                    # X^T slice (partitions 0-63) -> SBUF (exact f32 copy;
                    # the scalar engine's Copy path is low-precision)
                    xa_t = xap.tile([P_LOC, rt], dt.float32, tag="xat")
                    nc.vector.tensor_copy(xa_t, ps[0:D])

                    #   at[:, p] = sum_r e[p,r] * X^T[:, (p,r)], g[p] = sum_r e
                    tmp = tmpp.tile([P_LOC, rt], dt.float32, tag="tmp")
                    nc.vector.tensor_mul(tmp, xa_t, eb_t)
                    if t < NT - 1:
                        at_out = atA_sb[:, t * 9 : t * 9 + ipt]
                        g_out = gA_sb[0:1, t * 9 : t * 9 + ipt]
                    else:
                        at7 = finp.tile([P_LOC, 1], dt.float32, tag="at7")
                        g7 = finp.tile([1, 1], dt.float32, tag="g7")
                        at_out, g_out = at7, g7
                    nc.vector.tensor_reduce(
                        out=at_out,
                        in_=tmp.rearrange("p (i r) -> p i r", r=R),
                        axis=mybir.AxisListType.X,
                        op=mybir.AluOpType.add,
                    )
                    nc.vector.tensor_reduce(
                        out=g_out,
                        in_=eb_t[0:1, :].rearrange("a (i r) -> a i r", r=R),
                        axis=mybir.AxisListType.X,
                        op=mybir.AluOpType.add,
                    )

                    # item-level attention, batched: items 0-62 fire right
                    # after tile 6 (overlapping tile 7's matmuls); item 63's
                    # minimum-size chain is the only serial tail.
                    #   s = relu(E^T + A^T/(G*WSCALE)); l = s.Wi1
                    #   psv += exp(l) @ [p_pos | 1]
                    if t >= NT - 2:
                        first = t == NT - 2
                        nit = P_LOC - 1 if first else 1
                        at_in = atA_sb if first else at7
                        g_in = gA_sb if first else g7
                        rhs_pp = ppos_sb[0 : P_LOC - 1] if first else ppos63_sb
                        e_in = et_sb[:, 0 : P_LOC - 1] if first else et_sb[:, P_LOC - 1 : P_LOC]
                        ginv_t = finp.tile([1, nit], dt.float32, tag="ginv")
                        nc.vector.reciprocal(ginv_t, g_in)
                        psg = pssmall.tile([D, nit], dt.float32, tag="small")
                        nc.tensor.matmul(psg, ones32_sb[0:1, 0:D], ginv_t)
                        st_t = finp.tile([D, nit], dt.float32, tag="stt")
                        nc.vector.tensor_mul(st_t, at_in, psg)
                        nc.vector.tensor_add(st_t, st_t, e_in)
                        nc.vector.tensor_relu(st_t, st_t)
                        psa = pssmall.tile([nit, 1], dt.float32, tag="small")
                        nc.tensor.matmul(psa, st_t, wi1_sb)
                        ecol_t = finp.tile([nit, 1], dt.float32, tag="ecol")
                        nc.scalar.activation(ecol_t, psa, AF.Exp)
                        nc.tensor.matmul(
                            psv, ecol_t, rhs_pp, start=first, stop=not first
                        )

            out_sb = singles.tile([1, F + 1], dt.float32)
            nc.vector.tensor_copy(out_sb, psv)
            nc.sync.dma_start(out=out[:], in_=out_sb)

    nc.finalize()
    return nc


def _get_nc():
    if "nc" not in _CACHE:
        _CACHE["nc"] = _build()
    return _CACHE["nc"]


def kernel(**inputs) -> np.ndarray:
    f = np.asarray(inputs["f_u_i_pos"], dtype=np.float32)
    Gu = np.asarray(inputs["Gu"], np.float32)
    Gi = np.asarray(inputs["Gi"], np.float32)
    Pi = np.asarray(inputs["Pi"], np.float32)
    user = int(np.asarray(inputs["user"]))
    item = int(np.asarray(inputs["item"]))
    pos = np.asarray(inputs["user_pos"]).reshape(-1).astype(np.int64)

    g_u = Gu[user]
    gamma_i = Gi[item]
    p_i_item = Pi[item]
    g_pos = Gi[pos]
    p_pos = Pi[pos]

    uc = g_u @ np.asarray(inputs["Wc0_u"], np.float32) + np.asarray(inputs["bc0"], np.float32)
    ui = g_u @ np.asarray(inputs["Wi0_u"], np.float32) + np.asarray(inputs["bi0"], np.float32)
    E = (
        ui[None]
        + g_pos @ np.asarray(inputs["Wi0_iv"], np.float32)
        + p_pos @ np.asarray(inputs["Wi0_ip"], np.float32)
    )  # [512, 64]
    wc1 = np.asarray(inputs["Wc1"], np.float32).reshape(D, 1)
    wi1 = np.asarray(inputs["Wi1"], np.float32).reshape(D, 1)
    Wcat = np.concatenate(
        [np.asarray(inputs["Wi0_ix"], np.float32), np.asarray(inputs["Wc0_i"], np.float32)],
        axis=1,
    )  # [2048, 128]

    # Host packing: SBUF-layout-exact, so every device DMA is contiguous.
    import ml_dtypes

    f8 = ml_dtypes.float8_e4m3
    wcat_host = np.ascontiguousarray(
        (Wcat * WSCALE).reshape(KC, 128, 128).transpose(1, 0, 2).reshape(128, KC * 128)
    ).astype(f8)

    f8_rows = f[0].reshape(P_FULL * R, FEAT).astype(f8)  # [25088, 2048]

    in_maps = []
    for c in range(NCORES):
        fc_t = f8_rows[c * ROWS : (c + 1) * ROWS].T  # [2048, 3136] (view)
        a = fc_t.reshape(KC, 128, ROWS)
        ft_host = np.concatenate(
            [
                a[:, :, o : o + rt].transpose(1, 0, 2).reshape(128, KC * rt)
                for o, rt in zip(DMA_OFFS, DMA_RTS)
            ],
            axis=1,
        )  # [128, KC*ROWS], DMA-group-major, contiguous per partition slice
        ppos_ext = np.concatenate(
            [p_pos[c * P_LOC : (c + 1) * P_LOC], np.ones((P_LOC, 1), np.float32)], axis=1
        ).astype(np.float32)  # [64, 201]
        in_maps.append(
            {
                "ft": np.ascontiguousarray(ft_host),
                "wcat": wcat_host,
                "uc": uc.reshape(D, 1).astype(np.float32),
                "wc1t": np.ascontiguousarray(np.repeat(wc1, D, axis=1)).astype(np.float16),
                "et": np.ascontiguousarray(E[c * P_LOC : (c + 1) * P_LOC].T).astype(
                    np.float32
                ),
                "wi1": wi1,
                "ppos": ppos_ext,
            }
        )

    nc = _get_nc()
    _CACHE["in_maps"] = in_maps
    res = run_bass_kernel_spmd(nc, in_maps, core_ids=list(range(NCORES)))
    outs = [np.asarray(res.results[c]["out"][0], np.float64) for c in range(NCORES)]
    _CACHE["last_outs"] = outs

    V = sum(o[:F] for o in outs)
    S = sum(float(o[F]) for o in outs)
    all_a = V / S
    xui = np.float32(np.dot(g_u.astype(np.float64) + all_a, gamma_i.astype(np.float64)))
    return (np.array(xui, np.float32), g_u, gamma_i, p_i_item)


# revision 29
# speedup vs baseline: 1.2659x; 1.1428x over previous
"""Trainium2 Bass kernel for the ACF (Attentive Collaborative Filtering) model.

Strategy (8 NeuronCores, positive-item axis P=512 sharded 64 items/core):
  - The only heavy compute is f_u_i_pos @ [Wi0_ix | Wc0_i]  ([3136,2048]@[2048,128]
    per core).  f is shipped per-core as fp8-e4m3, pre-transposed and packed
    row-tile-major on the host so every DMA is contiguous and the contraction
    dim (features) lands on SBUF partitions.  Wcat is pre-scaled by 32 to dodge
    fp8 subnormals; the inverse scale is folded into the relu activation scale
    and the ginv broadcast constant (zero extra instructions).
  - One fused PE pass produces X^T = (f@Wi0_ix)^T on PSUM partitions 0-63 and
    H^T = (f@Wc0_i)^T on partitions 64-127.  Region softmax (over R=49) and the
    b-weighted sum over regions collapse to tiny per-core ops because
    all_x @ Wi0_ix == sum_r b*(f@Wi0_ix) -- all_x itself is never materialized.
  - Row tiles and epilogue tiles coincide (9 items per tile, plus a 1-item
    final tile so the unavoidable serial dependency chain at the end operates
    on minimum-size tensors).  The item-level attention (relu, Wi1 logit, exp,
    exp-weighted p_pos accumulation into a single PSUM bank) is also done
    per-tile, so it pipelines behind later tiles' matmuls.
  - Item-level attention partials (sum of exp, exp-weighted p_pos sum) are
    DMA'd out per core; the cross-shard softmax combine (exact: no max shift is
    used anywhere, and softmax is shift-invariant so bc1/bi1 cancel) and the
    final tiny dot products happen on the host during unsharding.
"""

import numpy as np

import concourse.bass as bass
import concourse.bacc as bacc
import concourse.tile as tile
from concourse import mybir
from concourse.bass_utils import run_bass_kernel_spmd

NCORES = 8
P_FULL = 512
R = 49
FEAT = 2048
D = 64
F = 200
P_LOC = P_FULL // NCORES      # 64 items per core
ROWS = P_LOC * R              # 3136 (item, region) rows per core
KC = FEAT // 128              # 16 contraction chunks of 128
RTS = [9 * R] * 7 + [R]       # row tiles (items*49): 7x441 + 1x49
NT = len(RTS)
OFFS = [sum(RTS[:i]) for i in range(len(RTS))]
DMA_RTS = [441, 882, 882, 882, 49]   # host packing granularity = DMA groups
DMA_OFFS = [sum(DMA_RTS[:i]) for i in range(len(DMA_RTS))]
WSCALE = 32.0                 # fp8 pre-scale on Wcat (folded back out downstream)

_CACHE: dict = {}


def _build():
    dt = mybir.dt
    AF = mybir.ActivationFunctionType
    nc = bacc.Bacc()

    ft = nc.declare_dram_parameter("ft", [128, KC * ROWS], dt.float8e4, isOutput=False)
    wcat = nc.declare_dram_parameter("wcat", [128, KC * 128], dt.float8e4, isOutput=False)
    uc = nc.declare_dram_parameter("uc", [D, 1], dt.float32, isOutput=False)
    wc1t = nc.declare_dram_parameter("wc1t", [D, D], dt.float16, isOutput=False)
    et = nc.declare_dram_parameter("et", [D, P_LOC], dt.float32, isOutput=False)
    wi1 = nc.declare_dram_parameter("wi1", [D, 1], dt.float32, isOutput=False)
    ppos = nc.declare_dram_parameter("ppos", [P_LOC, F + 1], dt.float32, isOutput=False)
    out = nc.declare_dram_parameter("out", [1, F + 1], dt.float32, isOutput=True)

    with tile.TileContext(nc) as tc:
        with (
            tc.tile_pool(name="singles", bufs=1) as singles,
            tc.tile_pool(name="ftp", bufs=3) as ftp,
            tc.tile_pool(name="eep", bufs=3) as eep,
            tc.tile_pool(name="xap", bufs=3) as xap,
            tc.tile_pool(name="ebp", bufs=3) as ebp,
            tc.tile_pool(name="hrp", bufs=2) as hrp,
            tc.tile_pool(name="tmpp", bufs=2) as tmpp,
            tc.tile_pool(name="finp", bufs=2) as finp,
            tc.tile_pool(name="psbig", bufs=2, space="PSUM") as psbig,
            tc.tile_pool(name="pssmall", bufs=2, space="PSUM") as pssmall,
            tc.tile_pool(name="psvp", bufs=1, space="PSUM") as psvp,
        ):
            # matmul #1 needs ft tile-0 quarter 0 and wcat chunks 0-3; order the
            # sync-queue DMAs so those land first.
            ft0 = ftp.tile([128, KC, RTS[0]], dt.float8e4, tag="ft0")
            q = KC // 4
            nc.sync.dma_start(out=ft0[:, 0:q], in_=ft[:, 0 : q * RTS[0]])
            wcat_sb = singles.tile([128, KC, 128], dt.float8e4)
            wcv = wcat.rearrange("p (k m) -> p k m", k=KC)
            nc.sync.dma_start(out=wcat_sb[:, 0:q], in_=wcv[:, 0:q])
            nc.sync.dma_start(out=wcat_sb[:, q:], in_=wcv[:, q:])
            for qi in range(1, 4):
                nc.sync.dma_start(
                    out=ft0[:, qi * q : (qi + 1) * q],
                    in_=ft[:, qi * q * RTS[0] : (qi + 1) * q * RTS[0]],
                )

            # small constants on the gpsimd (SWDGE) queue so they don't delay ft
            ucw_sb = singles.tile([128, 1], dt.float32)
            nc.gpsimd.dma_start(out=ucw_sb[D : 2 * D, :], in_=uc[:])
            wc1t_sb = singles.tile([128, D], dt.float16)
            nc.gpsimd.dma_start(out=wc1t_sb[D : 2 * D, :], in_=wc1t[:])
            et_sb = singles.tile([D, P_LOC], dt.float32)
            nc.gpsimd.dma_start(out=et_sb, in_=et[:])
            wi1_sb = singles.tile([D, 1], dt.float32)
            nc.gpsimd.dma_start(out=wi1_sb, in_=wi1[:])
            ppos_sb = singles.tile([P_LOC, F + 1], dt.float32)
            nc.gpsimd.dma_start(out=ppos_sb, in_=ppos[:])
            # item 63 (the 1-item last tile) needs its own base-0 copy
            ppos63_sb = singles.tile([1, F + 1], dt.float32)
            nc.gpsimd.dma_start(out=ppos63_sb, in_=ppos[P_LOC - 1 : P_LOC])

            ones16_sb = singles.tile([1, P_LOC], dt.float16)
            nc.vector.memset(ones16_sb, 1.0)
            ones32_sb = singles.tile([1, P_LOC], dt.float32)
            nc.vector.memset(ones32_sb, 1.0 / WSCALE)

            atA_sb = singles.tile([P_LOC, P_LOC - 1], dt.float32)
            gA_sb = singles.tile([1, P_LOC - 1], dt.float32)
            psv = psvp.tile([1, F + 1], dt.float32)

            # ft DMAs are merged in pairs of row tiles (bigger transfers keep
            # the 16 SDMA engines at line rate); processing stays per-tile.
            DMA_GROUPS = [[0], [1, 2], [3, 4], [5, 6], [7]]
            for grp in DMA_GROUPS:
                if grp == [0]:
                    ftg = ft0
                else:
                    w = sum(RTS[t] for t in grp)
                    ftg = ftp.tile([128, KC, w], dt.float8e4, tag="ftg")
                    nc.sync.dma_start(
                        out=ftg,
                        in_=ft[:, KC * OFFS[grp[0]] : KC * (OFFS[grp[0]] + w)],
                    )
                for t in grp:
                    rt = RTS[t]
                    lo = OFFS[t] - OFFS[grp[0]]
                    ipt = rt // R
                    ps = psbig.tile([128, rt], dt.float32, tag="ps")
                    for g in range(KC // 2):
                        nc.tensor.matmul(
                            ps,
                            wcat_sb[:, 2 * g : 2 * g + 2],
                            ftg[:, 2 * g : 2 * g + 2, lo : lo + rt],
                            start=(g == 0),
                            stop=(g == KC // 2 - 1),
                            perf_mode=mybir.MatmulPerfMode.DoubleRow,
                        )
                    # H^T slice (partitions 64-127): relu(H/WSCALE + uc), f16
                    hr = hrp.tile([128, rt], dt.float16, tag="hr")
                    nc.scalar.activation(
                        hr[D : 2 * D], ps[D : 2 * D], AF.Relu,
                        bias=ucw_sb[D : 2 * D, 0:1], scale=1.0 / WSCALE,
                    )
                    # region logits broadcast to 64 partitions in one matmul:
                    # wc1t is Wc1 replicated across 64 output columns, so every
                    # output partition carries the same logit row.  exp then
                    # lands already-broadcast (and runs on 64 ACT lanes).
                    psl = pssmall.tile([P_LOC, rt], dt.float32, tag="small")
                    nc.tensor.matmul(psl, wc1t_sb[D : 2 * D, :], hr[D : 2 * D])
                    eb_t = ebp.tile([P_LOC, rt], dt.float16, tag="ebt")
                    nc.scalar.activation(eb_t, psl, AF.Exp)

                    # X^T slice (partitions 0-63) -> SBUF (exact f32 copy;
                    # the scalar engine's Copy path is low-precision)
                    xa_t = xap.tile([P_LOC, rt], dt.float32, tag="xat")
                    nc.vector.tensor_copy(xa_t, ps[0:D])

                    #   at[:, p] = sum_r e[p,r] * X^T[:, (p,r)], g[p] = sum_r e
                    tmp = tmpp.tile([P_LOC, rt], dt.float32, tag="tmp")
                    nc.vector.tensor_mul(tmp, xa_t, eb_t)
                    if t < NT - 1:
                        at_out = atA_sb[:, t * 9 : t * 9 + ipt]
                        g_out = gA_sb[0:1, t * 9 : t * 9 + ipt]
                    else:
                        at7 = finp.tile([P_LOC, 1], dt.float32, tag="at7")
                        g7 = finp.tile([1, 1], dt.float32, tag="g7")
                        at_out, g_out = at7, g7
                    nc.vector.tensor_reduce(
                        out=at_out,
                        in_=tmp.rearrange("p (i r) -> p i r", r=R),
                        axis=mybir.AxisListType.X,
                        op=mybir.AluOpType.add,
                    )
                    nc.vector.tensor_reduce(
                        out=g_out,
                        in_=eb_t[0:1, :].rearrange("a (i r) -> a i r", r=R),
                        axis=mybir.AxisListType.X,
                        op=mybir.AluOpType.add,
                    )

                    # item-level attention, batched: items 0-62 fire right
                    # after tile 6 (overlapping tile 7's matmuls); item 63's
                    # minimum-size chain is the only serial tail.
                    #   s = relu(E^T + A^T/(G*WSCALE)); l = s.Wi1
                    #   psv += exp(l) @ [p_pos | 1]
                    if t >= NT - 2:
                        first = t == NT - 2
                        nit = P_LOC - 1 if first else 1
                        at_in = atA_sb if first else at7
                        g_in = gA_sb if first else g7
                        rhs_pp = ppos_sb[0 : P_LOC - 1] if first else ppos63_sb
                        e_in = et_sb[:, 0 : P_LOC - 1] if first else et_sb[:, P_LOC - 1 : P_LOC]
                        ginv_t = finp.tile([1, nit], dt.float32, tag="ginv")
                        nc.vector.reciprocal(ginv_t, g_in)
                        psg = pssmall.tile([D, nit], dt.float32, tag="small")
                        nc.tensor.matmul(psg, ones32_sb[0:1, 0:D], ginv_t)
                        st_t = finp.tile([D, nit], dt.float32, tag="stt")
                        nc.vector.tensor_mul(st_t, at_in, psg)
                        nc.vector.tensor_add(st_t, st_t, e_in)
                        nc.vector.tensor_relu(st_t, st_t)
                        psa = pssmall.tile([nit, 1], dt.float32, tag="small")
                        nc.tensor.matmul(psa, st_t, wi1_sb)
                        ecol_t = finp.tile([nit, 1], dt.float32, tag="ecol")
                        nc.scalar.activation(ecol_t, psa, AF.Exp)
                        nc.tensor.matmul(
                            psv, ecol_t, rhs_pp, start=first, stop=not first
                        )

            out_sb = singles.tile([1, F + 1], dt.float32)
            nc.vector.tensor_copy(out_sb, psv)
            nc.sync.dma_start(out=out[:], in_=out_sb)

    nc.finalize()
    return nc


def _get_nc():
    if "nc" not in _CACHE:
        _CACHE["nc"] = _build()
    return _CACHE["nc"]


def kernel(**inputs) -> np.ndarray:
    f = np.asarray(inputs["f_u_i_pos"], dtype=np.float32)
    Gu = np.asarray(inputs["Gu"], np.float32)
    Gi = np.asarray(inputs["Gi"], np.float32)
    Pi = np.asarray(inputs["Pi"], np.float32)
    user = int(np.asarray(inputs["user"]))
    item = int(np.asarray(inputs["item"]))
    pos = np.asarray(inputs["user_pos"]).reshape(-1).astype(np.int64)

    g_u = Gu[user]
    gamma_i = Gi[item]
    p_i_item = Pi[item]
    g_pos = Gi[pos]
    p_pos = Pi[pos]

    uc = g_u @ np.asarray(inputs["Wc0_u"], np.float32) + np.asarray(inputs["bc0"], np.float32)
    ui = g_u @ np.asarray(inputs["Wi0_u"], np.float32) + np.asarray(inputs["bi0"], np.float32)
    E = (
        ui[None]
        + g_pos @ np.asarray(inputs["Wi0_iv"], np.float32)
        + p_pos @ np.asarray(inputs["Wi0_ip"], np.float32)
    )  # [512, 64]
    wc1 = np.asarray(inputs["Wc1"], np.float32).reshape(D, 1)
    wi1 = np.asarray(inputs["Wi1"], np.float32).reshape(D, 1)
    Wcat = np.concatenate(
        [np.asarray(inputs["Wi0_ix"], np.float32), np.asarray(inputs["Wc0_i"], np.float32)],
        axis=1,
    )  # [2048, 128]

    # Host packing: SBUF-layout-exact, so every device DMA is contiguous.
    import ml_dtypes

    f8 = ml_dtypes.float8_e4m3
    wcat_host = np.ascontiguousarray(
        (Wcat * WSCALE).reshape(KC, 128, 128).transpose(1, 0, 2).reshape(128, KC * 128)
    ).astype(f8)

    f8_rows = f[0].reshape(P_FULL * R, FEAT).astype(f8)  # [25088, 2048]

    in_maps = []
    for c in range(NCORES):
        fc_t = f8_rows[c * ROWS : (c + 1) * ROWS].T  # [2048, 3136] (view)
        a = fc_t.reshape(KC, 128, ROWS)
        ft_host = np.concatenate(
            [
                a[:, :, o : o + rt].transpose(1, 0, 2).reshape(128, KC * rt)
                for o, rt in zip(DMA_OFFS, DMA_RTS)
            ],
            axis=1,
        )  # [128, KC*ROWS], DMA-group-major, contiguous per partition slice
        ppos_ext = np.concatenate(
            [p_pos[c * P_LOC : (c + 1) * P_LOC], np.ones((P_LOC, 1), np.float32)], axis=1
        ).astype(np.float32)  # [64, 201]
        in_maps.append(
            {
                "ft": np.ascontiguousarray(ft_host),
                "wcat": wcat_host,
                "uc": uc.reshape(D, 1).astype(np.float32),
                "wc1t": np.ascontiguousarray(np.repeat(wc1, D, axis=1)).astype(np.float16),
                "et": np.ascontiguousarray(E[c * P_LOC : (c + 1) * P_LOC].T).astype(
                    np.float32
                ),
                "wi1": wi1,
                "ppos": ppos_ext,
            }
        )

    nc = _get_nc()
    _CACHE["in_maps"] = in_maps
    res = run_bass_kernel_spmd(nc, in_maps, core_ids=list(range(NCORES)))
    outs = [np.asarray(res.results[c]["out"][0], np.float64) for c in range(NCORES)]
    _CACHE["last_outs"] = outs

    V = sum(o[:F] for o in outs)
    S = sum(float(o[F]) for o in outs)
    all_a = V / S
    xui = np.float32(np.dot(g_u.astype(np.float64) + all_a, gamma_i.astype(np.float64)))
    return (np.array(xui, np.float32), g_u, gamma_i, p_i_item)
